# revision 2
# baseline (speedup 1.0000x reference)
"""Trainium2 Bass kernel for nn_EquivariantProductBasisBlock — channel-major v2.

Computation (per node n, channel c):
  s = nf[n,c,0]; v = nf[n,c,1:4]; v2 = |v|^2
  out0 = w0*s + w1*s^2 + w2'*v2 + w3*s^3 + w4*s*v2       (w_p = W0[sp[n],p,c])
  B1   = u0 + u1'*s + u2'*s^2 + u3'*v2                   (u_p = W1[sp[n],p,c])
  out1m = B1 * v_m
  y0 = out0 @ L0 / sqrt(C);  y1m = out1m @ L1 / sqrt(C)
  y[n,c,:] = [y0, y1x, y1y, y1z] + sc[n,c,:]

Design (all mid-section bf16):
  - channel-major end to end: host ships nf/sc as per-channel planes
    [c, chunk, plane, n] bf16 so no deinterleave/transpose is ever needed
    on device; output returned channel-major and inverse-permuted on host.
  - per-node path weights via one-hot gather matmuls (K=10) writing bf16
    directly to PSUM; the Horner chain on DVE reads PSUM bf16 at 2x_1p.
  - channel mixing: lhsT = L (c on partitions), rhs = X' planes ->
    y^T planes in PSUM bf16; ACT copies to SBUF; sc added by accumulate-DMA.
  - engine split: DVE does the Horner/assembly ops; Pool (GpSimd) takes
    v2 adds + two cheap multiplies; ACT does squares + y copies.

Sharding: data-parallel over nodes across 8 cores (8192 nodes/core).
"""

import numpy as np

N_CORES = 8
N_NODES = 65536
C = 128
E = 10
NODES_PER_CORE = N_NODES // N_CORES          # 8192
B = 256                                       # nodes per chunk
N_CHUNKS = NODES_PER_CORE // B                # 32

INV_SQ3 = 1.0 / np.sqrt(3.0)
SQ2 = float(np.sqrt(2.0))
SQ3 = float(np.sqrt(3.0))
SQ35 = float(np.sqrt(3.0 / 5.0))

_CACHE = {}

# engine-assignment variant knobs (sim-tuned).
# NOTE: GPSIMD (Pool) cannot access PSUM on real HW -> ycopy engines are
# limited to "act"/"dve"/"dve_scadd".
VARIANT = {
    "ycopy_h0": "act",       # "act" | "dve" | "dve_scadd"
    "ycopy_h1": "dve",       # "act" | "dve" | "dve_scadd"
    "sq_engine": "pool",     # "dve" | "act" | "pool"
    "v2_engine": "pool",     # "pool" | "dve"
    "out1_engine": "pool",   # "pool" | "dve"
    # SBUF-only Horner ops optionally shifted to Pool:
    "pool_extra": ("B1a",),  # subset of {"hbm", "h3", "gv", "B1a"}
}


def _apply_patches():
    import concourse.tile as tile
    from concourse import mybir
    from concourse.vector_clock import ScopedClock

    if getattr(tile.TileContext, "_singlewait_patched", False):
        return

    def _patched_drain_and_barrier(self, tick_clock, wait_clock):
        nc = self.nc
        probe = nc.sync.nop()
        wait_clock.add_sem_waits(probe.ins, ScopedClock({None: tick_clock.global_clock}))
        si = probe.ins.sync_info
        waits = list(si.on_wait) if si and si.on_wait else []
        if len(waits) > 1:
            probe.ins.sync_info = type(si)(on_wait=waits[:1], on_update=[])
            for w in waits[1:]:
                extra = nc.sync.nop()
                extra.ins.sync_info = type(si)(on_wait=[w], on_update=[])
        nc.sync.drain()
        nc.all_engine_barrier()
        assert self.sems is not None
        popped = nc._tile_sem_poison_stack.pop()
        assert popped is self._sem_poison
        nc.clear_and_free_semaphores(list(self.sems.allocated().values()))
        nc.all_engine_barrier()

    _orig_commit = tile.TileContext._commit_instruction

    def _split_commit(self, inst, lazy_reg_writes=True):
        si = getattr(inst, "sync_info", None)
        if (si is not None and si.on_wait and len(si.on_wait) > 1
                and getattr(inst, "engine", mybir.EngineType.Unassigned)
                != mybir.EngineType.Unassigned):
            waits = list(si.on_wait)
            for w in waits[:-1]:
                nop = mybir.InstNoOp(name=self.nc.get_next_instruction_name(),
                                     ins=[], outs=[], engine=inst.engine)
                nop.sync_info = mybir.SyncInfo(on_wait=[w], on_update=[])
                _orig_commit(self, nop, lazy_reg_writes=False)
            inst.sync_info = mybir.SyncInfo(on_wait=[waits[-1]],
                                            on_update=list(si.on_update or []))
        return _orig_commit(self, inst, lazy_reg_writes)

    tile.TileContext._drain_and_barrier = _patched_drain_and_barrier
    tile.TileContext._commit_instruction = _split_commit
    tile.TileContext._singlewait_patched = True


def _build_program(reps=1, nodes=NODES_PER_CORE):
    import concourse.bass as bass
    import concourse.tile as tile
    from concourse import mybir
    from contextlib import ExitStack

    _apply_patches()
    BF16 = mybir.dt.bfloat16
    F32 = mybir.dt.float32
    nc = bass.Bass()

    n_super = nodes // (2 * B)               # super-chunks of 2*B nodes

    # channel-major DRAM layouts, 2 B/elem, contiguous 2*4*B per partition
    nf_d = nc.declare_dram_parameter("nf", [C, n_super, 2, 4, B], BF16, isOutput=False)
    sc_d = nc.declare_dram_parameter("sc", [C, n_super, 2, 4, B], BF16, isOutput=False)
    att_d = nc.declare_dram_parameter("att", [E, nodes], BF16, isOutput=False)
    w_d = nc.declare_dram_parameter("w01", [E, 1152], BF16, isOutput=False)
    l0_d = nc.declare_dram_parameter("l0", [C, C], BF16, isOutput=False)
    l1_d = nc.declare_dram_parameter("l1", [C, C], BF16, isOutput=False)
    out_d = nc.declare_dram_parameter("out", [C, n_super, 2, 4, B], BF16, isOutput=True)

    mult = mybir.AluOpType.mult
    add = mybir.AluOpType.add

    with tile.TileContext(nc) as tc, ExitStack() as ctx:
        consts = ctx.enter_context(tc.tile_pool(name="consts", bufs=1))
        chunks = ctx.enter_context(tc.tile_pool(name="chunks", bufs=3))
        work = ctx.enter_context(tc.tile_pool(name="work", bufs=2))
        psWA = ctx.enter_context(tc.tile_pool(name="psWA", bufs=1, space="PSUM"))
        psWB = ctx.enter_context(tc.tile_pool(name="psWB", bufs=1, space="PSUM"))
        psY = ctx.enter_context(tc.tile_pool(name="psY", bufs=1, space="PSUM"))

        # weight planes, column order: [w3|u2|w4] [w1|u1|w2] [w0|u0] [u3]
        t_w = consts.tile([E, 1152], BF16)
        nc.sync.dma_start(out=t_w, in_=w_d[:, :])
        t_l0 = consts.tile([C, C], BF16)
        nc.sync.dma_start(out=t_l0, in_=l0_d[:, :])
        t_l1 = consts.tile([C, C], BF16)
        nc.sync.dma_start(out=t_l1, in_=l1_d[:, :])
        t_att = consts.tile([E, nodes], BF16)
        nc.sync.dma_start(out=t_att, in_=att_d[:, :])

        def ap(t, off, *dims):
            return bass.AP(tensor=t.tensor, offset=t.offset + off,
                           ap=[t.ap[0], *list(dims)])

        H = 4 * B          # stride between halves in t_in/t_X/t_y
        W9 = 9 * B         # stride between halves in t_wS

        # software pipelining: the back-end (mix matmuls + y copies + sc/out
        # DMAs) of super-chunk k is emitted AFTER the front-end (gather +
        # staging + Horner) of super-chunk k+1, so the PE queue interleaves
        # gather(k+1) before mix(k) and never stalls on the DVE chain.
        pending_back = None

        def emit_back(cc, t_X):
            modes = (VARIANT["ycopy_h0"], VARIANT["ycopy_h1"])
            t_y = chunks.tile([C, 2, 4 * B], BF16, tag="y")
            t_sc = None
            if "dve_scadd" in modes:
                t_sc = chunks.tile([C, 2, 4 * B], BF16, tag="scin")
                nc.sync.dma_start(out=t_sc, in_=sc_d[:, cc])
            for h in range(2):
                p_y = psY.tile([C, 4 * B], F32, tag="py")
                for k in range(4):
                    nc.tensor.matmul(p_y[:, k * B:(k + 1) * B],
                                     lhsT=(t_l0 if k == 0 else t_l1),
                                     rhs=t_X[:, h, k * B:(k + 1) * B],
                                     start=(k % 2 == 0), stop=(k % 2 == 1))
                yeng = modes[h]
                if yeng == "act":
                    nc.scalar.copy(out=t_y[:, h], in_=p_y)
                elif yeng == "dve":
                    nc.vector.tensor_copy(out=t_y[:, h], in_=p_y)
                else:  # dve_scadd: fold the sc addition into the PSUM read
                    nc.vector.tensor_tensor(out=t_y[:, h], in0=p_y,
                                            in1=t_sc[:, h], op=add)
            # sc for plain-copied halves arrives via an accumulating DMA
            if modes[0] != "dve_scadd" and modes[1] != "dve_scadd":
                nc.gpsimd.dma_start(out=t_y, in_=sc_d[:, cc], accum_op=add)
            else:
                for h in range(2):
                    if modes[h] != "dve_scadd":
                        nc.gpsimd.dma_start(out=t_y[:, h], in_=sc_d[:, cc, h],
                                            accum_op=add)
            nc.sync.dma_start(out=out_d[:, cc], in_=t_y)

        for cc in [c for _ in range(reps) for c in range(n_super)]:
            t_in = chunks.tile([C, 2, 4 * B], BF16, tag="in")  # [h][s|vx|vy|vz]
            nc.sync.dma_start(out=t_in, in_=nf_d[:, cc])

            # both-halves views
            s1 = ap(t_in, 0, [H, 2], [1, B])
            v3 = ap(t_in, B, [H, 2], [1, 3 * B])
            s_b3 = ap(t_in, 0, [H, 2], [0, 3], [1, B])
            s_b2 = ap(t_in, 0, [H, 2], [0, 2], [1, B])

            # --- one-hot gather matmuls -> PSUM f32 (per half), staged to
            # SBUF bf16 by one wide ACT copy per half ---
            t_wS = work.tile([C, 2, 9 * B], BF16, tag="ws")
            for h in range(2):
                att_c = t_att[:, cc * 2 * B + h * B: cc * 2 * B + (h + 1) * B]
                # gather PSUM is split into two tiles (planes 0-4 / 5-8) so
                # the stage copy of one frees while the other gathers; two
                # 1 KB plane outputs share each 2 KB PSUM bank: the bank's
                # first writer carries start (wipes the whole zero region),
                # the second carries stop
                p_wA = psWA.tile([C, 5 * B], F32, tag="pwa")
                for p in range(5):
                    nc.tensor.matmul(p_wA[:, p * B:(p + 1) * B],
                                     lhsT=t_w[:, p * C:(p + 1) * C], rhs=att_c,
                                     start=(p % 2 == 0), stop=(p % 2 == 1 or p == 4))
                nc.scalar.copy(out=t_wS[:, h, 0:5 * B], in_=p_wA)
                p_wB = psWB.tile([C, 4 * B], F32, tag="pwb")
                for p in range(4):
                    nc.tensor.matmul(p_wB[:, p * B:(p + 1) * B],
                                     lhsT=t_w[:, (5 + p) * C:(6 + p) * C], rhs=att_c,
                                     start=(p % 2 == 0), stop=(p % 2 == 1))
                nc.scalar.copy(out=t_wS[:, h, 5 * B:9 * B], in_=p_wB)
            G_A = ap(t_wS, 0, [W9, 2], [1, 3 * B])         # [w3|u2|w4]
            G_B = ap(t_wS, 3 * B, [W9, 2], [1, 3 * B])     # [w1|u1|w2]
            G_C = ap(t_wS, 6 * B, [W9, 2], [1, 2 * B])     # [w0|u0]
            G_D = ap(t_wS, 8 * B, [W9, 2], [1, B])         # [u3]

            # --- v2 = |v|^2 ---
            sq_eng = {"dve": nc.vector, "act": None, "pool": nc.gpsimd}[VARIANT["sq_engine"]]
            t_sq = work.tile([C, 2, 3 * B], BF16, tag="sq")
            if sq_eng is None:
                nc.scalar.activation(out=t_sq, in_=v3,
                                     func=mybir.ActivationFunctionType.Square)
            else:
                sq_eng.tensor_tensor(out=t_sq, in0=v3, in1=v3, op=mult)
            v2_eng = nc.gpsimd if VARIANT["v2_engine"] == "pool" else nc.vector
            t_v2 = work.tile([C, 2, B], BF16, tag="v2")
            sq3 = 3 * B
            v2_eng.tensor_tensor(out=t_v2,
                                 in0=ap(t_sq, 0, [sq3, 2], [1, B]),
                                 in1=ap(t_sq, B, [sq3, 2], [1, B]), op=add)
            v2_eng.tensor_tensor(out=t_v2, in0=t_v2,
                                 in1=ap(t_sq, 2 * B, [sq3, 2], [1, B]), op=add)

            # --- Horner on DVE (staged bf16 SBUF operands run at 2x);
            # pool_extra ops shift to GpSimd ---
            px = VARIANT["pool_extra"]

            def eng(name):
                return nc.gpsimd if name in px else nc.vector

            T1 = work.tile([C, 2, 3 * B], BF16, tag="t1")
            nc.vector.tensor_tensor(out=T1, in0=s_b3, in1=G_A, op=mult)
            nc.vector.tensor_tensor(out=T1, in0=T1, in1=G_B, op=add)
            hb = ap(T1, 0, [sq3, 2], [1, 2 * B])
            eng("hbm").tensor_tensor(out=hb, in0=hb, in1=s_b2, op=mult)
            nc.vector.tensor_tensor(out=hb, in0=hb, in1=G_C, op=add)
            # T1 = [h2|b2|g] per half
            t_X = work.tile([C, 2, 4 * B], BF16, tag="x")  # [out0|o1x|o1y|o1z]
            t_h3 = work.tile([C, 2, B], BF16, tag="h3")
            eng("h3").tensor_tensor(out=t_h3,
                                    in0=ap(T1, 0, [sq3, 2], [1, B]),
                                    in1=s1, op=mult)
            t_gv = work.tile([C, 2, B], BF16, tag="gv")
            eng("gv").tensor_tensor(out=t_gv,
                                    in0=ap(T1, 2 * B, [sq3, 2], [1, B]),
                                    in1=t_v2, op=mult)
            nc.vector.tensor_tensor(out=ap(t_X, 0, [H, 2], [1, B]),
                                    in0=t_h3, in1=t_gv, op=add)
            t_q = work.tile([C, 2, B], BF16, tag="q")
            nc.vector.tensor_tensor(out=t_q, in0=G_D, in1=t_v2, op=mult)
            t_B1 = work.tile([C, 2, B], BF16, tag="b1")
            eng("B1a").tensor_tensor(out=t_B1,
                                     in0=ap(T1, B, [sq3, 2], [1, B]),
                                     in1=t_q, op=add)
            # out1 = B1 * v
            o1_eng = nc.gpsimd if VARIANT["out1_engine"] == "pool" else nc.vector
            o1_eng.tensor_tensor(out=ap(t_X, B, [H, 2], [1, 3 * B]),
                                 in0=ap(t_B1, 0, [B, 2], [0, 3], [1, B]),
                                 in1=v3, op=mult)

            # --- back-end of the PREVIOUS super-chunk ---
            if pending_back is not None:
                emit_back(*pending_back)
            pending_back = (cc, t_X)

        if pending_back is not None:
            emit_back(*pending_back)

    return nc


def _prep_host(inputs):
    import ml_dtypes
    bf16 = ml_dtypes.bfloat16

    nf = np.asarray(inputs["node_feats"], dtype=np.float32)
    sc = np.asarray(inputs["sc"], dtype=np.float32)
    sp = np.asarray(inputs["node_species"])
    W0 = np.asarray(inputs["W0"], dtype=np.float32)
    W1 = np.asarray(inputs["W1"], dtype=np.float32)
    L0 = np.asarray(inputs["L0"], dtype=np.float32)
    L1 = np.asarray(inputs["L1"], dtype=np.float32)

    att = (sp[None, :] == np.arange(E, dtype=sp.dtype)[:, None]).astype(bf16)

    w0 = W0.copy()
    w0[:, 2, :] *= INV_SQ3
    u = W1.copy()
    u[:, 1, :] *= SQ2
    u[:, 2, :] *= SQ3
    u[:, 3, :] *= SQ35
    # plane order: [w3|u2|w4] [w1|u1|w2] [w0|u0] [u3]
    w01 = np.concatenate([
        w0[:, 3, :], u[:, 2, :], w0[:, 4, :],
        w0[:, 1, :], u[:, 1, :], w0[:, 2, :],
        w0[:, 0, :], u[:, 0, :],
        u[:, 3, :],
    ], axis=1).astype(bf16)

    inv_sqrt_c = np.float32(1.0 / np.sqrt(C))
    l0 = np.ascontiguousarray(L0 * inv_sqrt_c).astype(bf16)
    l1 = np.ascontiguousarray(L1 * inv_sqrt_c).astype(bf16)

    # channel-major per-core planes: [core, c, chunk, plane, n]
    # nf: [n, c, 4] -> [c, n, m]; chunked along n
    nfT = nf.transpose(1, 0, 2).astype(bf16)     # [c, N, 4]
    scT = sc.transpose(1, 0, 2).astype(bf16)     # [c, N, 4]
    nf_planes = np.ascontiguousarray(
        nfT.reshape(C, N_CORES, N_CHUNKS, B, 4).transpose(1, 0, 2, 4, 3))
    sc_planes = np.ascontiguousarray(
        scT.reshape(C, N_CORES, N_CHUNKS, B, 4).transpose(1, 0, 2, 4, 3))
    return nf_planes, sc_planes, att, w01, l0, l1


def _in_maps(inputs):
    nf_planes, sc_planes, att, w01, l0, l1 = _prep_host(inputs)
    n_super = N_CHUNKS // 2
    maps = []
    for c in range(N_CORES):
        lo, hi = c * NODES_PER_CORE, (c + 1) * NODES_PER_CORE
        maps.append({
            "nf": nf_planes[c].reshape(C, n_super, 2, 4, B),
            "sc": sc_planes[c].reshape(C, n_super, 2, 4, B),
            "att": np.ascontiguousarray(att[:, lo:hi]),
            "w01": w01,
            "l0": l0,
            "l1": l1,
        })
    return maps


def _unpack_core(y):
    # [C, n_super, 2, 4, B] bf16 -> [n, c, 4] f32
    return (np.asarray(y, dtype=np.float32)
            .reshape(C, NODES_PER_CORE // B, 4, B)
            .transpose(1, 3, 0, 2)    # [chunk, n, c, m]
            .reshape(NODES_PER_CORE, C, 4))


def _unpack_out(res):
    return np.concatenate([_unpack_core(res.results[c]["out"])
                           for c in range(N_CORES)], axis=0)


def kernel(**inputs):
    from concourse.bass_utils import run_bass_kernel_spmd

    if "nc" not in _CACHE:
        _CACHE["nc"] = _build_program()
    nc = _CACHE["nc"]

    res = run_bass_kernel_spmd(nc, _in_maps(inputs), core_ids=list(range(N_CORES)),
                               **_CACHE.get("run_kwargs", {}))
    _CACHE["last_result"] = res
    return _unpack_out(res)
